# revision 1
# baseline (speedup 1.0000x reference)
"""Trainium2 Bass kernel for ive(63, z) = exp(-z) * I_63(z), elementwise over z[1048576].

Math: uniform asymptotic (Debye) expansion of log I_v(z) for fixed v=63:
    r = sqrt(z^2 + v^2),  Ls = ln(z^2 + v^2)
    log_ive = (r - z) + v*ln(z) - v*ln(v + r) - 0.25*Ls - 0.5*ln(2*pi) + q(Ls)
where q(Ls) ~= ln(1 + u1(t)/v + u2(t)/v^2), t = v*exp(-Ls/2), fit offline as a
cubic in Ls (max abs err 3.9e-5; the f32 reference's own noise vs truth is ~1e-4).

Sharding: pure elementwise; z split into 8 contiguous chunks of 131072 = [128,1024],
one per NeuronCore, no communication. Output flushed to 0 below FLT_MIN to match
the XLA reference (which produces no subnormals).
"""
import numpy as np
from contextlib import ExitStack

N = 1048576
NCORES = 8
P = 128
FD_CORE = 1024          # 128*1024 = 131072 elements per core
CHUNK = 512             # free-dim chunk per instruction
V2 = 3969.0             # 63^2

# q(Ls) cubic fit coeffs (power basis, c0..c3), fit over z in [0.1, 200]
C0 = -0.35607278238618795
C1 = 0.1060258802108635
C2 = -0.010491346839964302
C3 = 0.00034552829416086543
BIAS = C0 - 0.9189385332046727   # c0 + (-0.5*ln(2*pi)), folded into (r - z) op
C1F = C1 - 0.25                  # fold -0.25*Ls into the poly's linear term
FLT_MIN = 1.1754944e-38

_CACHE = {}


def _build():
    import concourse.tile as tile
    import concourse.mybir as mybir
    from concourse import bacc

    F32 = mybir.dt.float32
    AF = mybir.ActivationFunctionType
    ALU = mybir.AluOpType

    nc = bacc.Bacc("TRN2", target_bir_lowering=False, debug=False)
    # const APs for activation biases
    for v in (V2, 63.0, BIAS):
        t = nc.alloc_sbuf_tensor(f"constf32-{v}", [128, 1], F32)
        nc.gpsimd.memset(t.ap(), float(v))
        nc.const_aps.aps[(F32, float(v))] = t.ap()
    nc.all_engine_barrier()

    z = nc.dram_tensor("z", [P, FD_CORE], F32, kind="ExternalInput").ap()
    out = nc.dram_tensor("out", [P, FD_CORE], F32, kind="ExternalOutput").ap()

    with tile.TileContext(nc) as tc, ExitStack() as ctx:
        pool = ctx.enter_context(tc.tile_pool(name="sbuf", bufs=3))
        import concourse.bass as bass

        for i in range(FD_CORE // CHUNK):
            sl = bass.ts(i, CHUNK)
            zt = pool.tile([P, CHUNK], F32)
            nc.sync.dma_start(zt[:], z[:, sl])

            zsq = pool.tile([P, CHUNK], F32)
            nc.vector.tensor_tensor(zsq[:], zt[:], zt[:], ALU.mult)

            Ls = pool.tile([P, CHUNK], F32)
            nc.scalar.activation(Ls[:], zsq[:], AF.Ln, bias=V2, scale=1.0)
            r = pool.tile([P, CHUNK], F32)
            nc.scalar.activation(r[:], Ls[:], AF.Exp, bias=0.0, scale=0.5)
            L1 = pool.tile([P, CHUNK], F32)
            nc.scalar.activation(L1[:], zt[:], AF.Ln, bias=0.0, scale=1.0)
            L2 = pool.tile([P, CHUNK], F32)
            nc.scalar.activation(L2[:], r[:], AF.Ln, bias=63.0, scale=1.0)

            # q(Ls) + (-0.25*Ls): q0=C3*Ls; q1=(q0+C2)*Ls; q2=(q1+C1F)*Ls  (c0 -> BIAS)
            q0 = pool.tile([P, CHUNK], F32)
            nc.vector.tensor_scalar(q0[:], Ls[:], C3, None, ALU.mult)
            q1 = pool.tile([P, CHUNK], F32)
            nc.vector.scalar_tensor_tensor(q1[:], q0[:], C2, Ls[:], ALU.add, ALU.mult)
            q2 = pool.tile([P, CHUNK], F32)
            nc.vector.scalar_tensor_tensor(q2[:], q1[:], C1F, Ls[:], ALU.add, ALU.mult)

            # a = L1 - L2 (gpsimd); b = 63*a + q2; c = (r + BIAS) - z (gpsimd); g = b + c
            a = pool.tile([P, CHUNK], F32)
            nc.gpsimd.tensor_tensor(a[:], L1[:], L2[:], ALU.subtract)
            b = pool.tile([P, CHUNK], F32)
            nc.vector.scalar_tensor_tensor(b[:], a[:], 63.0, q2[:], ALU.mult, ALU.add)
            c = pool.tile([P, CHUNK], F32)
            nc.gpsimd.tensor_tensor(c[:], r[:], zt[:], ALU.subtract)
            g = pool.tile([P, CHUNK], F32)
            nc.vector.tensor_tensor(g[:], b[:], c[:], ALU.add)

            e = pool.tile([P, CHUNK], F32)
            nc.scalar.activation(e[:], g[:], AF.Exp, bias=BIAS, scale=1.0)
            # flush subnormals to 0 to match XLA: (e >= FLT_MIN) * e
            o = pool.tile([P, CHUNK], F32)
            nc.vector.scalar_tensor_tensor(o[:], e[:], FLT_MIN, e[:], ALU.is_ge, ALU.mult)

            nc.sync.dma_start(out[:, sl], o[:])

    nc.compile()
    return nc


def kernel(z: np.ndarray) -> np.ndarray:
    from concourse.bass_utils import run_bass_kernel_spmd

    if "nc" not in _CACHE:
        _CACHE["nc"] = _build()
    nc = _CACHE["nc"]

    z = np.ascontiguousarray(z, dtype=np.float32)
    zs = z.reshape(NCORES, P, FD_CORE)
    in_maps = [{"z": zs[i]} for i in range(NCORES)]
    res = run_bass_kernel_spmd(nc, in_maps, core_ids=list(range(NCORES)))
    out = np.stack([r["out"] for r in res.results])
    return out.reshape(N).astype(np.float32)



# revision 28
# speedup vs baseline: 1.2090x; 1.2090x over previous
"""Trainium2 Bass kernel for ive(63, z) = exp(-z) * I_63(z), elementwise over z[1048576].

Math: uniform asymptotic form of log I_v(z) for fixed v=63, with the Debye tail
folded into a cubic psi(r) evaluated via r^2 = zsq + 3969:
    r   = exp(0.5*ln(zsq + 3969))          [all activations stay in ONE
    Lq  = ln((63 + r)/z)                    act-table set: ln/exp/square/copy]
    log_ive = (r - z) - 63*Lq + C3*r^3 + C2*r^2 + C1*r + C0
            = r*(1 + C1 + 3969*C3 + C3*zsq) + C2*zsq - z - 63*Lq + (C0 + 3969*C2)
psi fit max abs err 3.2e-3 over z in [0.1, 200]; 1/z via the one-instruction
DVE approx reciprocal (~51 ULP).

Schedule: DVE/Pool/DMA ops run on SUB-wide slices; activations run ACTW-wide
(fewer instructions amortize the scalar engine's 185ns/instr SBUF latency).
Ops are emitted in dependency-depth wavefront order so each in-order engine
sequencer sees instructions in feasibility order. One pre-placed
LoadActFuncSet pins the ln+exp table (the auto-pass would otherwise reload
1283ns per activation phase). DMAs issue from the otherwise-idle SP engine.

Sharding: pure elementwise; z split into 8 contiguous chunks of 131072 = [128,1024],
one per NeuronCore, no communication. Output flushed to 0 below FLT_MIN to match
the XLA reference (which produces no subnormals).
"""
import numpy as np
from contextlib import ExitStack

N = 1048576
NCORES = 8
P = 128
FD_CORE = 1024          # 128*1024 = 131072 elements per core
SUB = 256               # DVE/Pool/DMA slice width
ACTW = 256              # activation instruction width; MUST equal SUB —
                        # wider act ops over narrower DMA slices race on HW
                        # (verified broken on device despite passing the sim)

# psi(r) = C0 + C1*r + C2*r^2 + C3*r^3 fit of log(ive) - (r-z) + 63*Lq
C0 = -2.3765373230792926
C1 = -0.012653454899332384
C2 = 5.0742094135462964e-05
C3 = -8.630069713366831e-08
T1B = 1.0 + C1 + 3969.0 * C3  # t1 = C3*zsq + T1B ; t2 = t1*r covers all r-terms
# folded const -> Exp bias; +2e-3 biases the flush boundary to the "keep"
# side so no element the reference keeps (>= FLT_MIN) gets flushed by our
# slightly-low estimate (a kept-extra element is benign for both metrics).
EBIAS = C0 + 3969.0 * C2 + 2e-3
FLT_MIN = 1.1754944e-38

_CACHE = {}


def _build(sub=SUB, actw=ACTW, dmaw=None, act_issue=False, flush=True):
    import concourse.tile as tile
    import concourse.mybir as mybir
    from concourse import bacc
    from concourse.hw_specs import get_activation_tables

    F32 = mybir.dt.float32
    AF = mybir.ActivationFunctionType
    ALU = mybir.AluOpType

    nc = bacc.Bacc("TRN2", target_bir_lowering=False, debug=False)

    z = nc.dram_tensor("z", [P, FD_CORE], F32, kind="ExternalInput").ap()
    out = nc.dram_tensor("out", [P, FD_CORE], F32, kind="ExternalOutput").ap()

    _set_id = list(get_activation_tables(nc.m.arch)).index(
        "natural_log_exp_and_others")
    nc.scalar.add_instruction(
        mybir.InstLoadActFuncSet(
            name=nc.get_next_instruction_name(),
            ins=[], outs=[], act_func_set_id=_set_id,
        )
    )

    ns = FD_CORE // sub            # sub-chunks
    spa = actw // sub              # sub-chunks per act chunk
    na = FD_CORE // actw           # act chunks
    if dmaw is None:
        dmaw = sub
    spd = dmaw // sub              # sub-chunks per dma chunk
    nd = FD_CORE // dmaw           # dma chunks

    with tile.TileContext(nc) as tc, ExitStack() as ctx:
        pool = ctx.enter_context(tc.tile_pool(name="sbuf", bufs=1))

        bias3969 = pool.tile([P, 1], F32, name="b3969")
        biasE = pool.tile([P, 1], F32, name="bE")
        nc.gpsimd.memset(bias3969[:], 3969.0)
        nc.gpsimd.memset(biasE[:], EBIAS)

        def full(nm):
            return pool.tile([P, FD_CORE], F32, name=nm)

        zt, w, zsq, Ls, r, rat, Lq = (full(x) for x in
                                      "zt w zsq Ls r rat Lq".split())
        t1, t2, t3, t4, t5, e, o = (full(x) for x in
                                    "t1 t2 t3 t4 t5 e o".split())

        def ssl(i):  # sub slice
            return slice(i * sub, (i + 1) * sub)

        def asl(a):  # act slice
            return slice(a * actw, (a + 1) * actw)

        sched = []  # (wavefront_key, phase_depth, order, fn)

        def add(key, depth, fn):
            sched.append((key, depth, len(sched), fn))

        # depths: dma0 w1 | zsq1 Ls2 r3 (act) | rat4 Lq5 (act) |
        #         t1:2 t2:4 t3:5 t4:6 t5:7 | e8 (act) | flush9 dma10
        def dsl(d):  # dma slice
            return slice(d * dmaw, (d + 1) * dmaw)

        for d in range(nd):
            eng = nc.scalar if (act_issue and d % 2 == 1) else nc.sync
            c = d * spd  # first covered sub-chunk gates the wavefront key
            add(c + 0, 0, lambda d=d, eng=eng: eng.dma_start(zt[:, dsl(d)],
                                                          z[:, dsl(d)]))
        for d in range(nd):
            c = d * spd + spd - 1  # ready when last covered flush lands
            add(c + 10, 10, lambda d=d: nc.sync.dma_start(out[:, dsl(d)],
                                                      (o if flush else e)[:, dsl(d)]))

        for i in range(ns):
            add(i + 1, 1, lambda i=i: nc.vector.reciprocal_approx_fast(
                out=w[:, ssl(i)], in_=zt[:, ssl(i)]))
            add(i + 4, 4, lambda i=i: nc.vector.scalar_tensor_tensor(
                rat[:, ssl(i)], r[:, ssl(i)], 63.0, w[:, ssl(i)],
                ALU.add, ALU.mult))
            add(i + 2, 2, lambda i=i: nc.vector.tensor_scalar(
                t1[:, ssl(i)], zsq[:, ssl(i)], C3, T1B, ALU.mult, ALU.add))
            add(i + 4, 4, lambda i=i: nc.gpsimd.tensor_tensor(
                t2[:, ssl(i)], t1[:, ssl(i)], r[:, ssl(i)], ALU.mult))
            add(i + 5, 5, lambda i=i: nc.vector.scalar_tensor_tensor(
                t3[:, ssl(i)], zsq[:, ssl(i)], C2, t2[:, ssl(i)],
                ALU.mult, ALU.add))
            add(i + 6, 6, lambda i=i: nc.vector.scalar_tensor_tensor(
                t4[:, ssl(i)], Lq[:, ssl(i)], -63.0, t3[:, ssl(i)],
                ALU.mult, ALU.add))
            add(i + 7, 7, lambda i=i: nc.gpsimd.tensor_tensor(
                t5[:, ssl(i)], t4[:, ssl(i)], zt[:, ssl(i)], ALU.subtract))
            if flush:
                add(i + 9, 9, lambda i=i: nc.vector.scalar_tensor_tensor(
                    o[:, ssl(i)], e[:, ssl(i)], FLT_MIN, e[:, ssl(i)],
                    ALU.is_ge, ALU.mult))

        for a in range(na):
            c = a * spa + spa - 1  # last covered sub-chunk
            add(c + 1, 1, lambda a=a: nc.scalar.activation(
                zsq[:, asl(a)], zt[:, asl(a)], AF.Square, bias=0.0, scale=1.0))
            add(c + 2, 2, lambda a=a: nc.scalar.activation(
                Ls[:, asl(a)], zsq[:, asl(a)], AF.Ln, bias=bias3969[:],
                scale=1.0))
            add(c + 3, 3, lambda a=a: nc.scalar.activation(
                r[:, asl(a)], Ls[:, asl(a)], AF.Exp, bias=0.0, scale=0.5))
            add(c + 5, 5, lambda a=a: nc.scalar.activation(
                Lq[:, asl(a)], rat[:, asl(a)], AF.Ln, bias=0.0, scale=1.0))
            add(c + 8, 8, lambda a=a: nc.scalar.activation(
                e[:, asl(a)], t5[:, asl(a)], AF.Exp, bias=biasE[:], scale=1.0))

        # wavefront order; deeper phases first within a wavefront
        sched.sort(key=lambda x: (x[0], -x[1], x[2]))
        for _, _, _, fn in sched:
            fn()

    nc.compile()
    return nc


def kernel(z: np.ndarray) -> np.ndarray:
    from concourse.bass_utils import run_bass_kernel_spmd

    if "nc" not in _CACHE:
        _CACHE["nc"] = _build()
    nc = _CACHE["nc"]

    z = np.ascontiguousarray(z, dtype=np.float32)
    zs = z.reshape(NCORES, P, FD_CORE)
    in_maps = [{"z": zs[i]} for i in range(NCORES)]
    res = run_bass_kernel_spmd(nc, in_maps, core_ids=list(range(NCORES)))
    out = np.stack([r["out"] for r in res.results])
    return out.reshape(N).astype(np.float32)



# revision 36
# speedup vs baseline: 1.2559x; 1.0388x over previous
"""Trainium2 Bass kernel for ive(63, z) = exp(-z) * I_63(z), elementwise over z[1048576].

Math: uniform asymptotic form of log I_v(z) for fixed v=63, with the Debye tail
folded into a cubic psi(r) evaluated via r^2 = zsq + 3969:
    r   = exp(0.5*ln(zsq + 3969))          [all activations stay in ONE
    Lq  = ln((63 + r)/z)                    act-table set: ln/exp/square/copy]
    log_ive = (r - z) - 63*Lq + C3*r^3 + C2*r^2 + C1*r + C0
            = r*(1 + C1 + 3969*C3 + C3*zsq) + C2*zsq - z - 63*Lq + (C0 + 3969*C2)
psi fit max abs err 3.2e-3 over z in [0.1, 200]; 1/z via the one-instruction
DVE approx reciprocal (~51 ULP).

Schedule: DVE/Pool/DMA ops run on SUB-wide slices; activations run ACTW-wide
(fewer instructions amortize the scalar engine's 185ns/instr SBUF latency).
Ops are emitted in dependency-depth wavefront order so each in-order engine
sequencer sees instructions in feasibility order. One pre-placed
LoadActFuncSet pins the ln+exp table (the auto-pass would otherwise reload
1283ns per activation phase). DMAs issue from the otherwise-idle SP engine.

Sharding: pure elementwise; z split into 8 contiguous chunks of 131072 = [128,1024],
one per NeuronCore, no communication. Output flushed to 0 below FLT_MIN to match
the XLA reference (which produces no subnormals).
"""
import numpy as np
from contextlib import ExitStack

N = 1048576
NCORES = 8
P = 128
FD_CORE = 1024          # 128*1024 = 131072 elements per core
SUB = 256               # DVE/Pool/DMA slice width
ACTW = 256              # activation instruction width; MUST equal SUB —
                        # wider act ops over narrower DMA slices race on HW
                        # (verified broken on device despite passing the sim)

# psi(r) = C0 + C1*r + C2*r^2 + C3*r^3 fit of log(ive) - (r-z) + 63*Lq
C0 = -2.3765373230792926
C1 = -0.012653454899332384
C2 = 5.0742094135462964e-05
C3 = -8.630069713366831e-08
T1B = 1.0 + C1 + 3969.0 * C3  # t1 = C3*zsq + T1B ; t2 = t1*r covers all r-terms
# folded const -> Exp bias; +2e-3 biases the flush boundary to the "keep"
# side so no element the reference keeps (>= FLT_MIN) gets flushed by our
# slightly-low estimate (a kept-extra element is benign for both metrics).
EBIAS = C0 + 3969.0 * C2 + 2e-3
FLT_MIN = 1.1754944e-38

_CACHE = {}


def _build(sub=SUB, actw=ACTW, dmaw=None, act_issue=False, flush=True,
           t5_dve=False, outw=None):
    import concourse.tile as tile
    import concourse.mybir as mybir
    from concourse import bacc
    from concourse.hw_specs import get_activation_tables

    F32 = mybir.dt.float32
    AF = mybir.ActivationFunctionType
    ALU = mybir.AluOpType

    nc = bacc.Bacc("TRN2", target_bir_lowering=False, debug=False)

    z = nc.dram_tensor("z", [P, FD_CORE], F32, kind="ExternalInput").ap()
    out = nc.dram_tensor("out", [P, FD_CORE], F32, kind="ExternalOutput").ap()

    _set_id = list(get_activation_tables(nc.m.arch)).index(
        "natural_log_exp_and_others")
    nc.scalar.add_instruction(
        mybir.InstLoadActFuncSet(
            name=nc.get_next_instruction_name(),
            ins=[], outs=[], act_func_set_id=_set_id,
        )
    )

    ns = FD_CORE // sub            # sub-chunks
    spa = actw // sub              # sub-chunks per act chunk
    na = FD_CORE // actw           # act chunks
    if dmaw is None:
        dmaw = sub
    spd = dmaw // sub              # sub-chunks per dma chunk
    nd = FD_CORE // dmaw           # dma chunks

    with tile.TileContext(nc) as tc, ExitStack() as ctx:
        pool = ctx.enter_context(tc.tile_pool(name="sbuf", bufs=1))

        bias3969 = pool.tile([P, 1], F32, name="b3969")
        biasE = pool.tile([P, 1], F32, name="bE")
        nc.gpsimd.memset(bias3969[:], 3969.0)
        nc.gpsimd.memset(biasE[:], EBIAS)

        def full(nm):
            return pool.tile([P, FD_CORE], F32, name=nm)

        zt, w, zsq, Ls, r, rat, Lq = (full(x) for x in
                                      "zt w zsq Ls r rat Lq".split())
        t1, t2, t3, t4, t5, e, o = (full(x) for x in
                                    "t1 t2 t3 t4 t5 e o".split())

        def ssl(i):  # sub slice
            return slice(i * sub, (i + 1) * sub)

        def asl(a):  # act slice
            return slice(a * actw, (a + 1) * actw)

        sched = []  # (wavefront_key, phase_depth, order, fn)

        def add(key, depth, fn):
            sched.append((key, depth, len(sched), fn))

        # depths: dma0 w1 | zsq1 (pool) Ls2 r3 | rat4 Lq5 |
        #         t1:2 t2:4 t3:5 t4:6 t5:7 | e8 | flush9 dma10
        def dsl(d):  # dma slice
            return slice(d * dmaw, (d + 1) * dmaw)

        for d in range(nd):
            eng = nc.scalar if (act_issue and d % 2 == 1) else nc.sync
            c = d * spd  # first covered sub-chunk gates the wavefront key
            add(c + 0, 0, lambda d=d, eng=eng: eng.dma_start(zt[:, dsl(d)],
                                                          z[:, dsl(d)]))
        ow = outw or dmaw
        spo = ow // sub
        for d in range(FD_CORE // ow):
            c = d * spo + spo - 1  # ready when last covered flush lands
            sl_ = slice(d * ow, (d + 1) * ow)
            add(c + 10, 10, lambda sl_=sl_: nc.sync.dma_start(
                out[:, sl_], (o if flush else e)[:, sl_]))

        for i in range(ns):
            add(i + 1, 1, lambda i=i: nc.vector.reciprocal_approx_fast(
                out=w[:, ssl(i)], in_=zt[:, ssl(i)]))
            # zsq = z*z on Pool (off the scalar engine, which was the wall)
            add(i + 1, 1, lambda i=i: nc.gpsimd.tensor_tensor(
                zsq[:, ssl(i)], zt[:, ssl(i)], zt[:, ssl(i)], ALU.mult))
            add(i + 4, 4, lambda i=i: nc.vector.scalar_tensor_tensor(
                rat[:, ssl(i)], r[:, ssl(i)], 63.0, w[:, ssl(i)],
                ALU.add, ALU.mult))
            add(i + 2, 2, lambda i=i: nc.vector.tensor_scalar(
                t1[:, ssl(i)], zsq[:, ssl(i)], C3, T1B, ALU.mult, ALU.add))
            add(i + 4, 4, lambda i=i: nc.gpsimd.tensor_tensor(
                t2[:, ssl(i)], t1[:, ssl(i)], r[:, ssl(i)], ALU.mult))
            add(i + 5, 5, lambda i=i: nc.vector.scalar_tensor_tensor(
                t3[:, ssl(i)], zsq[:, ssl(i)], C2, t2[:, ssl(i)],
                ALU.mult, ALU.add))
            add(i + 6, 6, lambda i=i: nc.vector.scalar_tensor_tensor(
                t4[:, ssl(i)], Lq[:, ssl(i)], -63.0, t3[:, ssl(i)],
                ALU.mult, ALU.add))
            if t5_dve:
                add(i + 7, 7, lambda i=i: nc.vector.scalar_tensor_tensor(
                    t5[:, ssl(i)], t4[:, ssl(i)], 0.0, zt[:, ssl(i)],
                    ALU.add, ALU.subtract))
            else:
                add(i + 7, 7, lambda i=i: nc.gpsimd.tensor_tensor(
                    t5[:, ssl(i)], t4[:, ssl(i)], zt[:, ssl(i)], ALU.subtract))
            if flush:
                add(i + 9, 9, lambda i=i: nc.vector.scalar_tensor_tensor(
                    o[:, ssl(i)], e[:, ssl(i)], FLT_MIN, e[:, ssl(i)],
                    ALU.is_ge, ALU.mult))

        for a in range(na):
            c = a * spa + spa - 1  # last covered sub-chunk
            add(c + 2, 2, lambda a=a: nc.scalar.activation(
                Ls[:, asl(a)], zsq[:, asl(a)], AF.Ln, bias=bias3969[:],
                scale=1.0))
            add(c + 3, 3, lambda a=a: nc.scalar.activation(
                r[:, asl(a)], Ls[:, asl(a)], AF.Exp, bias=0.0, scale=0.5))
            add(c + 5, 5, lambda a=a: nc.scalar.activation(
                Lq[:, asl(a)], rat[:, asl(a)], AF.Ln, bias=0.0, scale=1.0))
            add(c + 8, 8, lambda a=a: nc.scalar.activation(
                e[:, asl(a)], t5[:, asl(a)], AF.Exp, bias=biasE[:], scale=1.0))

        # wavefront order; deeper phases first within a wavefront
        sched.sort(key=lambda x: (x[0], -x[1], x[2]))
        for _, _, _, fn in sched:
            fn()

    nc.compile()
    return nc


def kernel(z: np.ndarray) -> np.ndarray:
    from concourse.bass_utils import run_bass_kernel_spmd

    if "nc" not in _CACHE:
        _CACHE["nc"] = _build()
    nc = _CACHE["nc"]

    z = np.ascontiguousarray(z, dtype=np.float32)
    zs = z.reshape(NCORES, P, FD_CORE)
    in_maps = [{"z": zs[i]} for i in range(NCORES)]
    res = run_bass_kernel_spmd(nc, in_maps, core_ids=list(range(NCORES)))
    out = np.stack([r["out"] for r in res.results])
    return out.reshape(N).astype(np.float32)

